# revision 6
# baseline (speedup 1.0000x reference)
"""Trainium2 Bass kernel for CMCAttn (channel attention x2 + cross attention).

Strategy (8 NeuronCores, pure data parallel): core = (batch b, query-half h).
Each core gets cnn_feat[b] (full, for channel-attn stats), its query half,
and vit_feat[b] (full), and computes its [2048, 4096] attention slab
flash-style (never materializing it in HBM).

Key algebraic folding: the channel-attention application
    x_att = gamma_cla * (attn_cc @ x) + x = (I + gamma_cla*attn_cc) @ x
is folded into the q/k/v projection weights on-device:
    lq = A_c^T Wq^T,  lk = A_v^T Wk^T,  R = [[A_v^T Wv^T, 0], [bv, 1]]
so the [C, N] attended features are never materialized. The v projection is
augmented with a ones column so the softmax denominator falls out of the
attn@v matmul (row 64 of out2 = rowsum of exp), and exp needs no max
subtraction for the cross attention (|energy| << 1 by construction); the
channel-attention softmax uses the exact shift exp(rowmin - e).
"""
import os
import sys

import numpy as np

if '/opt/trn_rl_repo' not in sys.path:
    sys.path.insert(0, '/opt/trn_rl_repo')

import concourse.bass as bass
import concourse.tile as tile
from concourse import bacc, mybir
from concourse.bass_utils import run_bass_kernel_spmd

B, C, H, W = 4, 64, 64, 64
N = H * W              # 4096
C8 = C // 8            # 8
NCORE = 8
QH = N // 2            # 2048 query rows per core
NKT = N // 128         # 32 k-tiles
F32 = mybir.dt.float32
AF = mybir.ActivationFunctionType
ALU = mybir.AluOpType


def _body(tc: tile.TileContext, t_in: dict, t_out):
    nc = tc.nc
    E = 1  # exp/eT dtype elements (fp32)

    with (
        tc.tile_pool(name="const", bufs=1) as cp,
        tc.tile_pool(name="data", bufs=1) as dp,
        tc.tile_pool(name="expp", bufs=2) as expp,
        tc.tile_pool(name="finp", bufs=2) as finp,
    ):
        # ---- constants / small weights to SBUF ----
        def cload(name, shape):
            t = cp.tile(shape, F32, tag=name)
            nc.sync.dma_start(t[:], t_in[name][:])
            return t

        wqT = cload('wqT', [C, C8])
        wkT = cload('wkT', [C, C8])
        wvT = cload('wvT', [C, C])
        eye = cload('eye64', [C, C])
        bq = cload('bq', [C8, 1])
        bk = cload('bk', [C8, 1])
        bvrow_d = t_in['bvrow']
        gcc = cload('gcc', [C, 1])
        gcv = cload('gcv', [C, 1])
        gmm = cload('gmm', [C, 1])
        ones64 = cp.tile([1, C], F32, tag="ones64")
        nc.gpsimd.memset(ones64[:], 1.0)

        # ---- big data tiles ----
        xfc = dp.tile([C, N], F32, tag="xfc")
        xfv = dp.tile([C + 1, N], F32, tag="xfv")      # row 64 = ones
        cnnh = dp.tile([C, QH], F32, tag="cnnh")
        for s in range(4):
            sl = slice(s * (N // 4), (s + 1) * (N // 4))
            nc.sync.dma_start(xfc[:, sl], t_in['cnn_full'][:, sl])
            nc.sync.dma_start(xfv[0:C, sl], t_in['vit_full'][:, sl])
        nc.gpsimd.memset(xfv[C:C + 1, :], 1.0)
        nc.sync.dma_start(cnnh[:], t_in['cnn_half'][:])

        xfcT = dp.tile([128, 2048], F32, tag="xfcT")
        xfvT = dp.tile([128, 2048], F32, tag="xfvT")
        qT_rep = dp.tile([128, QH], F32, tag="qT_rep")
        kT_tmp = dp.tile([C8, N], F32, tag="kT_tmp")
        kT_pk = dp.tile([128, 1024], F32, tag="kT_pk")
        v_sb = dp.tile([128, 65 * NKT], F32, tag="v_sb")

        # ================= phase 1: channel-attn stats =================
        def cl_stats(xf, xfT, gvec, nm):
            # transpose xf [64, 4096] -> xfT tiles [128 n, 64 c] (8 per bank)
            with tc.tile_pool(name=f"trp_{nm}", bufs=2, space="PSUM") as trp:
                for grp in range(4):
                    tp = trp.tile([128, 512], F32, tag="tr")
                    for u in range(8):
                        i = 8 * grp + u
                        nc.tensor.transpose(
                            tp[:, 64 * u:64 * (u + 1)],
                            xf[0:C, 128 * i:128 * (i + 1)], eye[:])
                    nc.vector.tensor_copy(
                        xfT[:, 512 * grp:512 * (grp + 1)], tp[:])
            with tc.tile_pool(name=f"eccp_{nm}", bufs=1, space="PSUM") as eccp:
                ecc = eccp.tile([C, C], F32, tag="ecc")
                for i in range(NKT):
                    sl = slice(64 * i, 64 * (i + 1))
                    nc.tensor.matmul(ecc[:], xfT[:, sl], xfT[:, sl],
                                     start=(i == 0), stop=(i == NKT - 1))
                minv = cp.tile([C, 1], F32, tag=f"minv_{nm}")
                nc.vector.tensor_reduce(minv[:], ecc[:],
                                        axis=mybir.AxisListType.X, op=ALU.min)
                expcc = cp.tile([C, C], F32, tag=f"expcc_{nm}")
                rsum = cp.tile([C, 1], F32, tag=f"rsum_{nm}")
                nc.scalar.activation(expcc[:], ecc[:], AF.Exp,
                                     bias=minv[:], scale=-1.0,
                                     accum_out=rsum[:])
            invs = cp.tile([C, 1], F32, tag=f"invs_{nm}")
            nc.vector.reciprocal(invs[:], rsum[:])
            attn = cp.tile([C, C], F32, tag=f"attn_{nm}")
            nc.vector.tensor_scalar_mul(attn[:], expcc[:], invs[:])
            gattn = cp.tile([C, C], F32, tag=f"gattn_{nm}")
            nc.vector.tensor_scalar_mul(gattn[:], attn[:], gvec[:])
            Ap = cp.tile([C, C], F32, tag=f"Ap_{nm}")
            nc.vector.tensor_add(Ap[:], gattn[:], eye[:])
            return Ap

        Ap_c = cl_stats(xfc, xfcT, gcc, "c")
        Ap_v = cl_stats(xfv, xfvT, gcv, "v")

        # ================= phase 2: folded projection weights ==========
        R = cp.tile([C + 1, C + 1], F32, tag="R")
        lq = cp.tile([C, C8], F32, tag="lq")
        lk = cp.tile([C, C8], F32, tag="lk")
        with tc.tile_pool(name="foldp", bufs=1, space="PSUM") as foldp:
            lqp = foldp.tile([C, C8], F32, tag="fold_q")
            nc.tensor.matmul(lqp[:], Ap_c[:], wqT[:], start=True, stop=True)
            nc.vector.tensor_copy(lq[:], lqp[:])
            lkp = foldp.tile([C, C8], F32, tag="fold_k")
            nc.tensor.matmul(lkp[:], Ap_v[:], wkT[:], start=True, stop=True)
            nc.vector.tensor_copy(lk[:], lkp[:])
            Rup = foldp.tile([C, C], F32, tag="fold_r")
            nc.tensor.matmul(Rup[:], Ap_v[:], wvT[:], start=True, stop=True)
            nc.vector.tensor_copy(R[0:C, 0:C], Rup[:])
        nc.gpsimd.memset(R[0:C, C:C + 1], 0.0)
        nc.sync.dma_start(R[C:C + 1, :], bvrow_d[:])

        # ================= phase 3: q/k/v projections ==================
        with (
            tc.tile_pool(name="qkp", bufs=2, space="PSUM") as qkp,
            tc.tile_pool(name="vp", bufs=3, space="PSUM") as vpp,
        ):
            for s in range(4):          # qT [8, 2048] (+bq), into row-group 0
                sl = slice(512 * s, 512 * (s + 1))
                qp = qkp.tile([C8, 512], F32, tag="qp")
                nc.tensor.matmul(qp[:], lq[:], cnnh[:, sl],
                                 start=True, stop=True)
                nc.vector.tensor_scalar_add(qT_rep[0:C8, sl], qp[:], bq[:])
            for s in range(8):          # kT [8, 4096] (+bk)
                sl = slice(512 * s, 512 * (s + 1))
                kp = qkp.tile([C8, 512], F32, tag="kp")
                nc.tensor.matmul(kp[:], lk[:], xfv[0:C, sl],
                                 start=True, stop=True)
                nc.vector.tensor_scalar_add(kT_tmp[:, sl], kp[:], bk[:])
            # replicate qT to row groups 32/64/96 for PE row-packing
            for g in range(1, 4):
                nc.sync.dma_start(qT_rep[32 * g:32 * g + C8, :],
                                  qT_rep[0:C8, :])
            # pack kT: group g holds k-tiles (4j+g), j=0..7, side by side
            ksrc = kT_tmp[:, :].rearrange("p (j f c) -> p j f c", j=8, f=4)
            kdst = kT_pk[:, :].rearrange("p (j c) -> p j c", j=8)
            for g in range(4):
                nc.sync.dma_start(kdst[32 * g:32 * g + C8, :, :],
                                  ksrc[:, :, g, :])
            # v_aug [n, 65] = xfv_aug^T @ R, 4 chunks per PSUM bank
            for grp in range(8):
                vp = vpp.tile([128, 4 * 65], F32, tag="vp")
                for u in range(4):
                    i = 4 * grp + u
                    nc.tensor.matmul(vp[:, 65 * u:65 * (u + 1)],
                                     xfv[:, 128 * i:128 * (i + 1)], R[:],
                                     start=True, stop=True)
                nc.vector.tensor_copy(
                    v_sb[:, 260 * grp:260 * (grp + 1)], vp[:])

        # ================= phase 4: main attention loop ================
        with (
            tc.tile_pool(name="eTp", bufs=1, space="PSUM") as eTp,
            tc.tile_pool(name="o2p", bufs=2, space="PSUM") as o2p,
            tc.tile_pool(name="ivp", bufs=2, space="PSUM") as ivp,
        ):
            for c in range(4):          # 512-wide q chunks
                qsl = slice(512 * c, 512 * (c + 1))
                o2 = o2p.tile([C + 1, 512], F32, tag="o2")
                for j in range(8):      # groups of 4 k-tiles (row-packed)
                    eT = eTp.tile([128, 2048], F32, tag="eT")
                    for g in range(4):
                        nc.tensor.matmul(
                            eT[:, 512 * g:512 * (g + 1)],
                            kT_pk[32 * g:32 * g + C8, 128 * j:128 * (j + 1)],
                            qT_rep[32 * g:32 * g + C8, qsl],
                            start=True, stop=True,
                            tile_position=(32 * g, 0))
                    ex = expp.tile([128, 2048], F32, tag="ex")
                    nc.scalar.activation(ex[:], eT[:], AF.Exp)
                    for g in range(4):
                        kt = 4 * j + g
                        nc.tensor.matmul(
                            o2[:], v_sb[:, 65 * kt:65 * (kt + 1)],
                            ex[:, 512 * g:512 * (g + 1)],
                            start=(j == 0 and g == 0),
                            stop=(j == 7 and g == 3),
                            skip_group_check=True)
                # finalize chunk c
                inv = finp.tile([1, 512], F32, tag="inv")
                nc.vector.reciprocal(inv[:], o2[C:C + 1, :])
                ivb = ivp.tile([C, 512], F32, tag="ivb")
                nc.tensor.matmul(ivb[:], ones64[:], inv[:],
                                 start=True, stop=True)
                o2g = finp.tile([C, 512], F32, tag="o2g")
                nc.scalar.activation(o2g[:], o2[0:C, :], AF.Identity,
                                     scale=gmm[:])
                prod = finp.tile([C, 512], F32, tag="prod")
                nc.vector.tensor_mul(prod[:], o2g[:], ivb[:])
                outf = finp.tile([C, 512], F32, tag="outf")
                nc.vector.tensor_add(outf[:], prod[:], cnnh[:, qsl])
                nc.sync.dma_start(t_out[:, qsl], outf[:])


_BUILT = {}


def _build(repeats=1):
    if repeats in _BUILT:
        return _BUILT[repeats]
    nc = bacc.Bacc("TRN2", target_bir_lowering=False, debug=False,
                   num_devices=NCORE)
    t_in = {
        'cnn_full': nc.dram_tensor('cnn_full', (C, N), F32,
                                   kind="ExternalInput"),
        'cnn_half': nc.dram_tensor('cnn_half', (C, QH), F32,
                                   kind="ExternalInput"),
        'vit_full': nc.dram_tensor('vit_full', (C, N), F32,
                                   kind="ExternalInput"),
        'wqT': nc.dram_tensor('wqT', (C, C8), F32, kind="ExternalInput"),
        'wkT': nc.dram_tensor('wkT', (C, C8), F32, kind="ExternalInput"),
        'wvT': nc.dram_tensor('wvT', (C, C), F32, kind="ExternalInput"),
        'eye64': nc.dram_tensor('eye64', (C, C), F32, kind="ExternalInput"),
        'bq': nc.dram_tensor('bq', (C8, 1), F32, kind="ExternalInput"),
        'bk': nc.dram_tensor('bk', (C8, 1), F32, kind="ExternalInput"),
        'bvrow': nc.dram_tensor('bvrow', (1, C + 1), F32,
                                kind="ExternalInput"),
        'gcc': nc.dram_tensor('gcc', (C, 1), F32, kind="ExternalInput"),
        'gcv': nc.dram_tensor('gcv', (C, 1), F32, kind="ExternalInput"),
        'gmm': nc.dram_tensor('gmm', (C, 1), F32, kind="ExternalInput"),
    }
    t_out = nc.dram_tensor('out', (C, QH), F32, kind="ExternalOutput")
    with tile.TileContext(nc) as tc:
        for _ in range(repeats):
            _body(tc, t_in, t_out[:])
    nc.compile()
    _BUILT[repeats] = nc
    return nc


def _make_in_maps(inputs):
    cnn = np.ascontiguousarray(
        np.asarray(inputs['cnn_feat'], np.float32).reshape(B, C, N))
    vit = np.ascontiguousarray(
        np.asarray(inputs['vit_feat'], np.float32).reshape(B, C, N))
    f32 = lambda x: np.ascontiguousarray(np.asarray(x, np.float32))
    wqT = f32(np.asarray(inputs['Wq'], np.float32).T)
    wkT = f32(np.asarray(inputs['Wk'], np.float32).T)
    wvT = f32(np.asarray(inputs['Wv'], np.float32).T)
    eye = np.eye(C, dtype=np.float32)
    bq = f32(inputs['bq']).reshape(C8, 1)
    bk = f32(inputs['bk']).reshape(C8, 1)
    bvrow = np.concatenate(
        [np.asarray(inputs['bv'], np.float32),
         np.ones(1, np.float32)]).reshape(1, C + 1)
    gcc = np.full((C, 1), np.float32(np.asarray(inputs['gamma_cla_cnn']).reshape(-1)[0]), np.float32)
    gcv = np.full((C, 1), np.float32(np.asarray(inputs['gamma_cla_vit']).reshape(-1)[0]), np.float32)
    gmm = np.full((C, 1), np.float32(np.asarray(inputs['gamma']).reshape(-1)[0]), np.float32)
    in_maps = []
    for core in range(NCORE):
        b, h = core // 2, core % 2
        in_maps.append({
            'cnn_full': cnn[b],
            'cnn_half': np.ascontiguousarray(cnn[b][:, h * QH:(h + 1) * QH]),
            'vit_full': vit[b],
            'wqT': wqT, 'wkT': wkT, 'wvT': wvT, 'eye64': eye,
            'bq': bq, 'bk': bk, 'bvrow': bvrow,
            'gcc': gcc, 'gcv': gcv, 'gmm': gmm,
        })
    return in_maps


def _run(inputs, repeats=1, **kwargs):
    nc = _build(repeats)
    res = run_bass_kernel_spmd(nc, _make_in_maps(inputs),
                               core_ids=list(range(NCORE)), **kwargs)
    out = np.empty((B, C, N), np.float32)
    for core in range(NCORE):
        b, h = core // 2, core % 2
        out[b][:, h * QH:(h + 1) * QH] = res.results[core]['out']
    return out.reshape(B, C, H, W), res


def kernel(**inputs) -> np.ndarray:
    out, _ = _run(inputs)
    return out
